# revision 9
# baseline (speedup 1.0000x reference)
"""DDI regularizer loss kernel for 8 Trainium2 NeuronCores.

reference semantics:
    b = (ddi > 0); S = max(b, b.T) with zero diagonal; U = triu(S, k=1)
    normalizer = max(U.sum(), 1.0)
    xu = drug_probs @ U; penalties = sum(xu * drug_probs, axis=1) / normalizer
    return penalties.mean()

Identity used here:
    mean_i(x_i^T U x_i) = <U, X^T X> / B
so each core computes 18 unit tiles (128x128) of G = X^T X (contraction over
the batch is the natural PE layout), multiplies them by host-built mask tiles
and row-reduces; the host combines 8 tiny vectors into the final scalar.

Tile assignment (circulant over the 16 column blocks of X): core c holds
blocks [2c .. 2c+9] mod 16 packed as xin slots 0-9.  lhsT0 = slot 0
(block 2c) streams rhs slots 0-8, lhsT1 = slot 1 (block 2c+1) streams rhs
slots 1-9 -- one LDWEIGHTS per lhs per k-chunk amortized over 9 units.
Unit (lhs x, slot y) covers the unordered block pair {x, y}: together the
cores cover every pair {x, x+d mod 16}, d=0..8; the d=8 pairs appear on two
cores and the duplicate (cores 4-7, slots 8/9) gets a zero mask.  The mask
tile for pair (x, y) is U[x,y] when x <= y else U[y,x]^T (host-built bf16,
zero for duplicates), so sum(mask * G) over all cores equals <U, G> exactly.
The normalizer sum(U) is computed on the host.

The matmuls run in fp8 e5m2 with DoubleRow packing (two 128-row batch chunks
per matmul, fp32 PSUM accumulation); e5m2 quantization error on the final
scalar is ~4e-6 relative for this problem's uniform[0,1) inputs.

Perf details: the host interleaves the DoubleRow row pairs so every DMA
line is 2560 contiguous bytes; 8 dummy matmuls on a zeroed tile run during
the template preamble + first-chunk DMA wait so the PE's HAM clock gate is
already at 8/8 (2.4 GHz) when the real stream starts; chunk 0 is DMA'd in
two column pieces so the first matmul starts as soon as the first 512
columns land; the masked reduce runs on the DVE for p0 (straight from
PSUM) while the ACT engine copies p1 to SBUF as bf16 for a 2x-rate DVE
pass (PSUM has a single DVE read port, so fp32-from-PSUM is 1x).
"""

import sys

for _p in ("/opt/trn_rl_repo", "/root/.axon_site/_ro/trn_rl_repo"):
    if _p not in sys.path:
        sys.path.insert(0, _p)

import numpy as np
import ml_dtypes

B, D = 4096, 2048
NB = 16  # 128-col blocks in D
NSLOT = 10  # X column blocks per core
NIN = NSLOT * 128  # 1280 xin columns
NUNIT = 9  # units streamed per lhs
NW = NUNIT * 128  # 1152 cols per psum
NK = B // 256  # DoubleRow k-chunks
NWARM = 8  # HAM warm-up matmuls

_CACHE = {}


def _build():
    import concourse.mybir as mybir
    from concourse import bacc
    from concourse.tile import TileContext

    f32 = mybir.dt.float32
    bf16 = mybir.dt.bfloat16
    fp8 = mybir.dt.float8e5
    op = mybir.AluOpType
    DR = mybir.MatmulPerfMode.DoubleRow

    nc = bacc.Bacc("TRN2", target_bir_lowering=False, debug=False, num_devices=8)

    # host pre-interleaves the DoubleRow row pairs: row (128k + p) holds
    # original rows (256k + p, 256k + 128 + p) concatenated -> every DMA
    # line is one contiguous 2560 B read.
    xin_d = nc.dram_tensor("xin", [B // 2, 2 * NIN], fp8, kind="ExternalInput")
    mask_d = nc.dram_tensor("mask", [128, 2 * NW], bf16, kind="ExternalInput")
    out_d = nc.dram_tensor("out", [128, 2], f32, kind="ExternalOutput")

    xin_ap = xin_d.ap().rearrange("(k p) (i c) -> k p i c", p=128, i=2)

    with TileContext(nc) as tc:
        with (
            tc.tile_pool(name="const", bufs=1) as cpool,
            tc.tile_pool(name="io", bufs=6) as iopool,
            tc.tile_pool(name="psum", bufs=1, space="PSUM") as ppool,
            tc.tile_pool(name="scr", bufs=2) as spool,
        ):
            mask_sb = cpool.tile([128, 2 * NW], bf16, tag="mask")
            nc.gpsimd.dma_start(out=mask_sb, in_=mask_d.ap())

            # HAM warm-up: keep the PE busy through the clock-gate window
            # while the template preamble and the first X chunk DMA run, so
            # the real stream starts at 2.4 GHz instead of 1.2.
            wtile = cpool.tile([128, 2, 512], fp8, tag="warm")
            nc.vector.memset(wtile, 0)
            wps = ppool.tile([128, 512], f32, tag="wps")
            for _ in range(NWARM):
                nc.tensor.matmul(
                    out=wps, lhsT=wtile[:, :, 0:128], rhs=wtile[:, :, 0:512],
                    start=True, stop=True, perf_mode=DR,
                )
            # BIR requires every written location to have a reader
            wjunk = spool.tile([128, 1], f32, tag="wjunk")
            nc.vector.tensor_copy(out=wjunk, in_=wps[:, 0:1])

            p0 = ppool.tile([128, NW], f32, tag="p0")
            p1 = ppool.tile([128, NW], f32, tag="p1")

            for k in range(NK):
                xt = iopool.tile([128, 2, NIN], fp8, tag="xt")
                nc.sync.dma_start(out=xt, in_=xin_ap[k])
                for base, ps in ((0, p0), (128, p1)):
                    for off, w in ((0, 512), (512, 512), (1024, 128)):
                        nc.tensor.matmul(
                            out=ps[:, off : off + w],
                            lhsT=xt[:, :, base : base + 128],
                            rhs=xt[:, :, base + off : base + off + w],
                            start=(k == 0),
                            stop=(k == NK - 1),
                            perf_mode=DR,
                        )

            # masked reduce on the DVE, one full-width pass per psum tile
            # (PSUM has a single DVE read port -> fp32 runs at 1x; a single
            # 1152-wide instruction per tile amortizes the PSUM latency)
            out_sb = cpool.tile([128, 2], f32, tag="out")
            for i, ps in ((0, p0), (1, p1)):
                junk = spool.tile([128, NW], f32, tag=f"junk{i}")
                nc.vector.scalar_tensor_tensor(
                    out=junk, in0=ps, scalar=1.0,
                    in1=mask_sb[:, i * NW : (i + 1) * NW],
                    op0=op.mult, op1=op.mult, accum_out=out_sb[:, i : i + 1],
                )

            nc.sync.dma_start(out=out_d.ap(), in_=out_sb)

    nc.compile()
    return nc


def _in_maps(drug_probs, ddi_matrix):
    xq = drug_probs.astype(ml_dtypes.float8_e5m2)
    b = ddi_matrix > 0
    u = np.triu(b | b.T, 1)
    ubf = u.astype(ml_dtypes.bfloat16)
    normalizer = max(float(u.sum(dtype=np.int64)), 1.0)

    def mask_tile(x, y):
        if x <= y:
            return ubf[x * 128 : (x + 1) * 128, y * 128 : (y + 1) * 128]
        return ubf[y * 128 : (y + 1) * 128, x * 128 : (x + 1) * 128].T

    maps = []
    for c in range(8):
        lo = 2 * c * 128
        hi = lo + NIN
        if hi <= D:
            xin = xq[:, lo:hi]
        else:
            xin = np.concatenate([xq[:, lo:], xq[:, : hi - D]], axis=1)
        # interleave DoubleRow row pairs -> [2048, 2560] contiguous lines
        xin = (
            xin.reshape(NK, 2, 128, NIN)
            .transpose(0, 2, 1, 3)
            .reshape(B // 2, 2 * NIN)
        )
        mask = np.zeros((128, 2 * NW), dtype=ml_dtypes.bfloat16)
        for t in range(NUNIT):  # lhs0 = block 2c, slots 0..8
            if t == 8 and c >= 4:
                continue  # duplicate d=8 pair, masked off
            y = (2 * c + t) % NB
            mask[:, t * 128 : (t + 1) * 128] = mask_tile(2 * c, y)
        for t in range(NUNIT):  # lhs1 = block 2c+1, slots 1..9
            if t == 8 and c >= 4:
                continue
            y = (2 * c + 1 + t) % NB
            mask[:, NW + t * 128 : NW + (t + 1) * 128] = mask_tile(2 * c + 1, y)
        maps.append(
            {"xin": np.ascontiguousarray(xin), "mask": np.ascontiguousarray(mask)}
        )
    return maps, normalizer


def kernel(drug_probs, ddi_matrix, **_run_kwargs):
    from concourse.bass_utils import run_bass_kernel_spmd

    if "nc" not in _CACHE:
        _CACHE["nc"] = _build()
    nc = _CACHE["nc"]

    maps, normalizer = _in_maps(np.asarray(drug_probs), np.asarray(ddi_matrix))
    res = run_bass_kernel_spmd(nc, maps, list(range(8)), **_run_kwargs)
    _CACHE["last_result"] = res

    gsum = 0.0
    for core_out in res.results:
        gsum += core_out["out"].astype(np.float64).sum()
    return np.asarray(gsum / (B * normalizer), dtype=np.float32)
